# revision 6
# baseline (speedup 1.0000x reference)
"""Trainium2 Bass kernel for nn_Encoder_44736379355603 (2-layer GCN encoder).

Math (PyG GCNConv with self-loops, eval mode):
    deg = in-degree over (edges + self-loops); dis = deg^-1/2
    g1 = dis * (x @ W1)                       # [N, 64] table, replicated per core
    h  = relu(dis * A_sum(g1) + b1)           # A_sum = segment_sum over in-edges
    g2 = dis * (h @ [W_mu | W_ls])            # [N, 32], AllGather across cores
    mu|ls = dis * A_sum(g2) + [b_mu | b_ls]

Device mapping (8 NeuronCores, single SPMD program):
  - dst nodes sharded: core c owns rows [c*12544, (c+1)*12544) of the padded
    100352-node space.
  - Per core, edges grouped in 4 src-windows (25088 nodes) so gather indices
    fit int16 for dma_gather; within a stream, edges sorted by dst and padded
    so each (dst-window, src-window) range has identical column counts on all
    cores (uniform program; per-core variation lives in data).
  - Gather: dma_gather chunks (<=4096 256B rows) round-robin on 4 SWDGE
    queues. Layer 2 gathers 256B node-PAIRS from the 32-channel g2 table;
    parity selection is folded into the one-hot.
  - Segment-sum: per 128-dst window, one-hot matrices built on DVE via
    is_equal(iota, dstid) feed PE matmuls accumulating in PSUM.
  - Between layers: on-device AllGather of the per-core g2 shard.
"""
import numpy as np

P = 128
CH1 = 64          # layer-1 channels (gather row = 256B)
CH2 = 32          # layer-2 channels (mu|ls)
GCHUNK = 4096     # rows per dma_gather instruction
OHB = 4           # one-hot builds batched per DVE op
N_CORES = 8


class Plan:
    """Host-side graph preprocessing producing uniform per-core schedules."""

    def __init__(self, n_nodes, n_cores, edge_src, edge_dst):
        self.n_cores = n_cores
        shard = -(-n_nodes // (n_cores * P)) * P
        npad = shard * n_cores
        win_src = npad // 4
        self.shard, self.npad, self.win_src = shard, npad, win_src
        self.nwin = shard // P
        assert win_src % 2 == 0 and win_src - 1 < 32768 and win_src // 2 - 1 < 32768

        deg = np.bincount(edge_dst, minlength=n_nodes) + 1
        self.dis = np.zeros(npad, np.float32)
        self.dis[:n_nodes] = (1.0 / np.sqrt(deg.astype(np.float64))).astype(np.float32)

        loops = np.arange(n_nodes, dtype=np.int64)
        src = np.concatenate([edge_src, loops])
        dst = np.concatenate([edge_dst, loops])

        core = dst // shard
        w = (dst % shard) // P
        q = src // win_src
        order = np.lexsort((dst, q, core))
        src, dst, core, q, w = src[order], dst[order], core[order], q[order], w[order]

        cnt = np.zeros((n_cores, 4, self.nwin), np.int64)
        np.add.at(cnt, (core, q, w), 1)
        self.pwq = (-(-cnt.max(axis=0) // P) * P).astype(np.int64)   # [4, nwin]
        self.stream_off = np.zeros((4, self.nwin), np.int64)
        for qq in range(4):
            self.stream_off[qq] = np.cumsum(self.pwq[qq]) - self.pwq[qq]
        self.stream_len = self.pwq.sum(axis=1)
        self.chunks = []
        for qq in range(4):
            L, ch = int(self.stream_len[qq]), []
            while L > 0:
                s = min(GCHUNK, L)
                ch.append(s)
                L -= s
            self.chunks.append(ch)

        self.per_core = []
        for c in range(n_cores):
            m = core == c
            sc, dc, qc, wc = src[m], dst[m], q[m], w[m]
            run_key = qc * self.nwin + wc
            starts = np.flatnonzero(np.diff(run_key, prepend=-1))
            rank = np.arange(len(run_key)) - np.repeat(
                np.arange(len(run_key))[starts],
                np.diff(starts, append=len(run_key)))
            pos = self.stream_off[qc, wc] + rank
            self.per_core.append((sc, dc, qc, wc, pos))

        tot_cols = int(self.pwq.sum()) // P
        self.ne1 = -(-tot_cols // OHB) * OHB
        self.ne2 = -(-(2 * tot_cols) // OHB) * OHB

    def core_arrays(self, c):
        """idx streams (int16) and schedule-ordered dstid arrays for core c."""
        sc, dc, qc, wc, pos = self.per_core[c]
        shard, win_src = self.shard, self.win_src
        idx1, idx2, d1s, d2es, d2os = [], [], [], [], []
        for q in range(4):
            L = int(self.stream_len[q])
            i1 = np.zeros(L, np.int16)
            i2 = np.zeros(L, np.int16)
            d1 = np.full(L, -1.0, np.float32)
            d2e = np.full(L, -1.0, np.float32)
            d2o = np.full(L, -1.0, np.float32)
            m = qc == q
            p_, s_, d_ = pos[m], sc[m], dc[m]
            i1[p_] = (s_ - q * win_src).astype(np.int16)
            i2[p_] = ((s_ >> 1) - q * (win_src // 2)).astype(np.int16)
            local = (d_ - c * shard - wc[m] * P).astype(np.float32)
            d1[p_] = local
            even = (s_ % 2) == 0
            d2e[p_[even]] = local[even]
            d2o[p_[~even]] = local[~even]
            idx1.append(i1); idx2.append(i2)
            d1s.append(d1); d2es.append(d2e); d2os.append(d2o)

        a1, a2 = [], []
        for w in range(self.nwin):
            for q in range(4):
                o = int(self.stream_off[q, w])
                n = int(self.pwq[q, w])
                a1.append(d1s[q][o:o + n].reshape(-1, P))
                se = d2es[q][o:o + n].reshape(-1, P)
                so = d2os[q][o:o + n].reshape(-1, P)
                a2.append(np.stack([se, so], axis=1).reshape(-1, P))
        a1 = np.concatenate(a1)
        a2 = np.concatenate(a2)
        a1 = np.concatenate([a1, np.full((self.ne1 - len(a1), P), -1.0, np.float32)])
        a2 = np.concatenate([a2, np.full((self.ne2 - len(a2), P), -1.0, np.float32)])
        return idx1, idx2, a1.T.copy(), a2.T.copy()


def _wrap16(a):
    n = a.shape[0]
    assert n % 16 == 0
    w = a.reshape(n // 16, 16).T.astype(np.int16)
    return np.tile(w, (8, 1))


def build_program(plan, n_cores, rep=1, n_ag=1):
    """rep>1 builds a timing variant: n_ag AllGathers up front, then the full
    pipeline (minus AllGather) repeated `rep` times inside a For_i loop."""
    import concourse.bass as bass
    import concourse.bacc as bacc
    import concourse.mybir as mybir
    import concourse.tile as tile

    shard, npad, nwin, win_src = plan.shard, plan.npad, plan.nwin, plan.win_src
    f32 = mybir.dt.float32
    i16 = mybir.dt.int16
    Copy = mybir.ActivationFunctionType.Copy

    nc = bacc.Bacc("TRN2", target_bir_lowering=False, debug=False,
                   enable_asserts=False, num_devices=n_cores,
                   num_swdge_queues=4)

    xT = nc.dram_tensor("xT", [P, npad], f32, kind="ExternalInput")
    W1 = nc.dram_tensor("W1", [P, CH1], f32, kind="ExternalInput")
    Wc = nc.dram_tensor("Wc", [CH1, CH2], f32, kind="ExternalInput")
    b1r = nc.dram_tensor("b1r", [P, CH1], f32, kind="ExternalInput")
    bcr = nc.dram_tensor("bcr", [P, CH2], f32, kind="ExternalInput")
    iot = nc.dram_tensor("iot", [P, P], f32, kind="ExternalInput")
    idn = nc.dram_tensor("idn", [P, P], f32, kind="ExternalInput")
    disT = nc.dram_tensor("disT", [P, npad // P], f32, kind="ExternalInput")
    disW = nc.dram_tensor("disW", [P, nwin], f32, kind="ExternalInput")
    idx1 = [nc.dram_tensor(f"idx1_{q}", [P, int(plan.stream_len[q]) // 16], i16,
                           kind="ExternalInput") for q in range(4)]
    idx2 = [nc.dram_tensor(f"idx2_{q}", [P, int(plan.stream_len[q]) // 16], i16,
                           kind="ExternalInput") for q in range(4)]
    did1 = nc.dram_tensor("did1", [P, plan.ne1], f32, kind="ExternalInput")
    did2 = nc.dram_tensor("did2", [P, plan.ne2], f32, kind="ExternalInput")
    out_c = nc.dram_tensor("out_c", [shard, CH2], f32, kind="ExternalOutput")

    g1w = [nc.dram_tensor(f"g1w{q}", [win_src, CH1], f32, kind="Internal")
           for q in range(4)]
    g2c = nc.dram_tensor("g2c", [shard, CH2], f32, kind="Internal")
    g2f = nc.dram_tensor("g2f", [npad, CH2], f32, kind="Internal")

    GB = 512
    n_groups = win_src // GB
    assert win_src % GB == 0

    with tile.TileContext(nc) as tc:
        with (
            tc.tile_pool(name="const", bufs=1) as cpool,
            tc.tile_pool(name="xload", bufs=1) as xpool,
            tc.tile_pool(name="stage", bufs=1) as spool,
            tc.tile_pool(name="gat", bufs=1) as gpool,
            tc.tile_pool(name="oh", bufs=1) as ohpool,
            tc.tile_pool(name="didp", bufs=1) as dpool,
            tc.tile_pool(name="ixp", bufs=1) as ipool,
            tc.tile_pool(name="epi", bufs=1) as epool,
            tc.tile_pool(name="ps", bufs=1, space="PSUM") as pspool,
        ):
            W1_t = cpool.tile([P, CH1], f32, name="W1_t")
            Wc_t = cpool.tile([CH1, CH2], f32, name="Wc_t")
            b1_t = cpool.tile([P, CH1], f32, name="b1_t")
            bc_t = cpool.tile([P, CH2], f32, name="bc_t")
            io_t = cpool.tile([P, P], f32, name="io_t")
            id_t = cpool.tile([P, P], f32, name="id_t")
            dT_t = cpool.tile([P, npad // P], f32, name="dT_t")
            dW_t = cpool.tile([P, nwin], f32, name="dW_t")
            for t, d in ((W1_t, W1), (Wc_t, Wc), (b1_t, b1r), (bc_t, bcr),
                         (io_t, iot), (id_t, idn), (dT_t, disT), (dW_t, disW)):
                nc.sync.dma_start(out=t[:], in_=d[:, :])

            g2f_pairs = g2f[:, :].rearrange("(a b) c -> a (b c)", b=2)

            def emit_g1_build():
                for q in range(4):
                    for g in range(n_groups):
                        base = q * win_src + g * GB
                        xt = xpool.tile([P, GB], f32, tag="xt", bufs=3, name="xt")
                        nc.sync.dma_start(out=xt[:], in_=xT[:, base:base + GB])
                        st = spool.tile([P, (GB // P) * CH1], f32, tag="st",
                                        bufs=3, name="st")
                        for s in range(GB // P):
                            ps = pspool.tile([P, CH1], f32, tag="gx", bufs=2,
                                             name="gx")
                            nc.tensor.matmul(out=ps[:],
                                             lhsT=xt[:, s * P:(s + 1) * P],
                                             rhs=W1_t[:], start=True, stop=True)
                            col = (base + s * P) // P
                            nc.scalar.activation(
                                out=st[:, s * CH1:(s + 1) * CH1], in_=ps[:],
                                func=Copy, scale=dT_t[:, col:col + 1])
                        nc.sync.dma_start(
                            out=g1w[q][g * GB:(g + 1) * GB, :].rearrange(
                                "(s p) c -> p s c", p=P),
                            in_=st[:].rearrange("p (s c) -> p s c", c=CH1))

            def propagate(layer):
                idxs = idx1 if layer == 1 else idx2
                did = did1 if layer == 1 else did2
                n_ent = plan.ne1 if layer == 1 else plan.ne2
                gt_cur = [None] * 4
                ch_next = [0] * 4
                ch_start = [0] * 4
                dstate = {"e0": -(10 ** 9), "tile": None, "oh": None}

                def get_gather(q, pos):
                    while True:
                        cur = gt_cur[q]
                        if cur is not None and cur[0] <= pos < cur[1]:
                            return cur
                        s0 = ch_start[q]
                        n = plan.chunks[q][ch_next[q]]
                        ix = ipool.tile([P, GCHUNK // 16], i16,
                                        tag=f"ix{layer}{q}", bufs=4,
                                        name=f"ix{layer}{q}")
                        nc.sync.dma_start(out=ix[:, :n // 16],
                                          in_=idxs[q][:, s0 // 16:(s0 + n) // 16])
                        gt = gpool.tile([P, (GCHUNK // P) * CH1], f32,
                                        tag=f"g{q}", bufs=4, name=f"g{q}")
                        src_ap = g1w[q][:] if layer == 1 else \
                            g2f_pairs[q * (win_src // 2):(q + 1) * (win_src // 2), :]
                        nc.gpsimd.dma_gather(
                            out_ap=gt[:, :(n // P) * CH1].rearrange(
                                "p (n c) -> p n c", c=CH1),
                            in_ap=src_ap, idxs_ap=ix[:, :n // 16],
                            num_idxs=n, num_idxs_reg=n, elem_size=CH1,
                            single_packet=False, queue_num=q)
                        gt_cur[q] = (s0, s0 + n, gt)
                        ch_start[q] += n
                        ch_next[q] += 1

                def get_oh(entry):
                    if entry % OHB == 0:
                        if not (dstate["e0"] <= entry < dstate["e0"] + 128):
                            e0 = entry - entry % 128
                            dt = dpool.tile([P, 128], f32, tag=f"dt{layer}",
                                            bufs=3, name=f"dt{layer}")
                            nl = min(128, n_ent - e0)
                            nc.sync.dma_start(out=dt[:, :nl],
                                              in_=did[:, e0:e0 + nl])
                            dstate["e0"], dstate["tile"] = e0, dt
                        dt = dstate["tile"]
                        k = entry - dstate["e0"]
                        oh = ohpool.tile([P, OHB * P], f32, tag="oh", bufs=6,
                                         name="oh")
                        iota_b = io_t[:].rearrange(
                            "p (a c) -> p a c", a=1).to_broadcast([P, OHB, P])
                        did_b = dt[:, k:k + OHB].rearrange(
                            "p (b c) -> p b c", c=1).to_broadcast([P, OHB, P])
                        nc.vector.tensor_tensor(
                            out=oh[:].rearrange("p (b c) -> p b c", c=P),
                            in0=iota_b, in1=did_b, op=mybir.AluOpType.is_equal)
                        dstate["oh"] = oh
                    k = entry % OHB
                    return dstate["oh"][:, k * P:(k + 1) * P]

                entry = 0
                for w in range(nwin):
                    ps = pspool.tile([P, CH1], f32, tag="win", bufs=3, name="win")
                    n_mm = (int(plan.pwq[:, w].sum()) // P) * (2 if layer == 2 else 1)
                    mm = 0
                    for q in range(4):
                        base_pos = int(plan.stream_off[q, w])
                        for cc in range(int(plan.pwq[q, w]) // P):
                            pos = base_pos + cc * P
                            s0, _s1, gt = get_gather(q, pos)
                            j = (pos - s0) // P
                            if layer == 1:
                                oh = get_oh(entry); entry += 1
                                nc.tensor.matmul(
                                    out=ps[:], lhsT=oh,
                                    rhs=gt[:, j * CH1:(j + 1) * CH1],
                                    start=(mm == 0), stop=(mm == n_mm - 1))
                                mm += 1
                            else:
                                ohe = get_oh(entry); entry += 1
                                oho = get_oh(entry); entry += 1
                                nc.tensor.matmul(
                                    out=ps[:, :CH2], lhsT=ohe,
                                    rhs=gt[:, j * CH1:j * CH1 + CH2],
                                    start=(mm == 0), stop=False)
                                mm += 1
                                nc.tensor.matmul(
                                    out=ps[:, :CH2], lhsT=oho,
                                    rhs=gt[:, j * CH1 + CH2:(j + 1) * CH1],
                                    start=False, stop=(mm == n_mm - 1))
                                mm += 1
                    yield w, ps

            def emit_l1():
                for w, ps in propagate(1):
                    t1 = epool.tile([P, CH1], f32, tag="t1", bufs=3, name="t1")
                    nc.scalar.activation(out=t1[:], in_=ps[:], func=Copy,
                                         scale=dW_t[:, w:w + 1])
                    t2 = epool.tile([P, CH1], f32, tag="t2", bufs=3, name="t2")
                    nc.vector.tensor_tensor(out=t2[:], in0=t1[:], in1=b1_t[:],
                                            op=mybir.AluOpType.add)
                    h = epool.tile([P, CH1], f32, tag="h", bufs=3, name="h")
                    nc.vector.tensor_scalar_max(out=h[:], in0=t2[:], scalar1=0.0)
                    pt = pspool.tile([CH1, P], f32, tag="tp", bufs=1, name="tp")
                    nc.tensor.transpose(out=pt[:], in_=h[:], identity=id_t[:])
                    hT = epool.tile([CH1, P], f32, tag="hT", bufs=3, name="hT")
                    nc.vector.tensor_copy(out=hT[:], in_=pt[:])
                    pg = pspool.tile([P, CH2], f32, tag="mm2", bufs=2, name="mm2")
                    nc.tensor.matmul(out=pg[:], lhsT=hT[:], rhs=Wc_t[:],
                                     start=True, stop=True)
                    g2s = epool.tile([P, CH2], f32, tag="g2s", bufs=3, name="g2s")
                    nc.scalar.activation(out=g2s[:], in_=pg[:], func=Copy,
                                         scale=dW_t[:, w:w + 1])
                    nc.sync.dma_start(out=g2c[w * P:(w + 1) * P, :], in_=g2s[:])

            def emit_ag():
                nc.gpsimd.collective_compute(
                    "AllGather", mybir.AluOpType.bypass,
                    replica_groups=[list(range(n_cores))],
                    ins=[g2c[:, :]], outs=[g2f[:, :]])

            def emit_l2():
                for w, ps in propagate(2):
                    o1 = epool.tile([P, CH2], f32, tag="o1", bufs=3, name="o1")
                    nc.scalar.activation(out=o1[:], in_=ps[:, :CH2], func=Copy,
                                         scale=dW_t[:, w:w + 1])
                    o2 = epool.tile([P, CH2], f32, tag="o2", bufs=3, name="o2")
                    nc.vector.tensor_tensor(out=o2[:], in0=o1[:], in1=bc_t[:],
                                            op=mybir.AluOpType.add)
                    nc.sync.dma_start(out=out_c[w * P:(w + 1) * P, :], in_=o2[:])

            if rep == 1:
                emit_g1_build()
                emit_l1()
                emit_ag()
                emit_l2()
            else:
                for _ in range(n_ag):
                    emit_ag()
                with tc.For_i(0, rep, 1) as _i:
                    emit_g1_build()
                    emit_l1()
                    emit_l2()

    nc.compile()
    return nc


def make_in_maps(plan, x, W1, b1, W_mu, b_mu, W_ls, b_ls):
    n_nodes = np.asarray(x).shape[0]
    npad, shard = plan.npad, plan.shard
    xTf = np.zeros((P, npad), np.float32)
    xTf[:, :n_nodes] = np.asarray(x, np.float32).T
    Wc = np.concatenate([np.asarray(W_mu, np.float32),
                         np.asarray(W_ls, np.float32)], axis=1)
    bc = np.concatenate([np.asarray(b_mu, np.float32),
                         np.asarray(b_ls, np.float32)])
    iota = np.tile(np.arange(P, dtype=np.float32), (P, 1))
    ident = np.eye(P, dtype=np.float32)
    disT = plan.dis.reshape(npad // P, P).T.copy()

    in_maps = []
    for c in range(plan.n_cores):
        i1, i2, a1, a2 = plan.core_arrays(c)
        m = {
            "xT": xTf, "W1": np.asarray(W1, np.float32), "Wc": Wc,
            "b1r": np.tile(np.asarray(b1, np.float32), (P, 1)),
            "bcr": np.tile(bc, (P, 1)), "iot": iota, "idn": ident,
            "disT": disT,
            "disW": plan.dis[c * shard:(c + 1) * shard].reshape(
                plan.nwin, P).T.copy(),
            "did1": np.ascontiguousarray(a1),
            "did2": np.ascontiguousarray(a2),
        }
        for q in range(4):
            m[f"idx1_{q}"] = _wrap16(i1[q])
            m[f"idx2_{q}"] = _wrap16(i2[q])
        in_maps.append(m)
    return in_maps


def kernel(x, edge_index, W1, b1, W_mu, b_mu, W_ls, b_ls):
    from concourse import bass_utils

    x = np.asarray(x, np.float32)
    n_nodes = x.shape[0]
    plan = Plan(n_nodes, N_CORES, np.asarray(edge_index[0], np.int64),
                np.asarray(edge_index[1], np.int64))
    nc = build_program(plan, N_CORES)
    in_maps = make_in_maps(plan, x, W1, b1, W_mu, b_mu, W_ls, b_ls)
    res = bass_utils.run_bass_kernel_spmd(nc, in_maps,
                                          core_ids=list(range(N_CORES)))
    out = np.concatenate([res.results[c]["out_c"] for c in range(N_CORES)],
                         axis=0)
    return (out[:n_nodes, :16].copy(), out[:n_nodes, 16:].copy())


def _numpy_ref(x, ei, W1, b1, W_mu, b_mu, W_ls, b_ls):
    n = x.shape[0]
    src = np.concatenate([ei[0], np.arange(n)])
    dst = np.concatenate([ei[1], np.arange(n)])
    deg = np.bincount(dst, minlength=n)
    dis = 1 / np.sqrt(deg)

    def conv(f, W, b):
        g = dis[:, None] * (f @ W)
        acc = np.zeros((n, W.shape[1]))
        np.add.at(acc, dst, g[src])
        return dis[:, None] * acc + b

    h = np.maximum(conv(x, W1, b1), 0)
    return conv(h, W_mu, b_mu), conv(h, W_ls, b_ls)


if __name__ == "__main__":
    rng = np.random.default_rng(0)
    N, E, IC = 2048, 16384, 128
    x = rng.standard_normal((N, IC)).astype(np.float32)
    ei = rng.integers(0, N, size=(2, E)).astype(np.int64)
    W1 = (rng.standard_normal((IC, CH1)) / np.sqrt(IC)).astype(np.float32)
    b1 = (rng.standard_normal(CH1) * 0.1).astype(np.float32)
    W_mu = (rng.standard_normal((CH1, 16)) / 8).astype(np.float32)
    b_mu = (rng.standard_normal(16) * 0.1).astype(np.float32)
    W_ls = (rng.standard_normal((CH1, 16)) / 8).astype(np.float32)
    b_ls = (rng.standard_normal(16) * 0.1).astype(np.float32)

    emu, els = _numpy_ref(x, ei, W1, b1, W_mu, b_mu, W_ls, b_ls)
    amu, als = kernel(x, ei, W1, b1, W_mu, b_mu, W_ls, b_ls)
    for name, e, a in (("mu", emu, amu), ("ls", els, als)):
        rel = np.abs(a - e).max() / (np.abs(e).max() + 1e-9)
        print(f"{name}: rel {rel:.3e}")


# revision 12
# speedup vs baseline: 4.0608x; 4.0608x over previous
"""Trainium2 Bass kernel for nn_Encoder_44736379355603 (2-layer GCN encoder).

Math (PyG GCNConv with self-loops, eval mode):
    deg = in-degree over (edges + self-loops); dis = deg^-1/2
    g1 = dis * (x @ W1)                       # [N, 64] table, replicated per core
    h  = relu(dis * A_sum(g1) + b1)           # A_sum = segment_sum over in-edges
    g2 = dis * (h @ [W_mu | W_ls])            # [N, 32], AllGather across cores
    mu|ls = dis * A_sum(g2) + [b_mu | b_ls]

Device mapping (8 NeuronCores, single SPMD program):
  - dst nodes sharded: core c owns rows [c*12544, (c+1)*12544) of the padded
    100352-node space.
  - Per core, edges grouped in 4 src-windows (25088 nodes) so gather indices
    fit int16 for dma_gather; within a stream, edges sorted by dst and padded
    so each (dst-window, src-window) range has identical column counts on all
    cores (uniform program; per-core variation lives in data).
  - Gather: dma_gather chunks (<=4096 256B rows) round-robin on 4 SWDGE
    queues. Layer 2 gathers 256B node-PAIRS from the 32-channel g2 table;
    parity selection is folded into the one-hot.
  - Segment-sum: per 128-dst window, one-hot matrices built on DVE via
    is_equal(iota, dstid) feed PE matmuls accumulating in PSUM.
  - Between layers: on-device AllGather of the per-core g2 shard.
"""
import numpy as np

P = 128
CH1 = 64          # layer-1 channels (gather row = 256B)
CH2 = 32          # layer-2 channels (mu|ls)
GCHUNK = 4096     # rows per dma_gather instruction
OHB = 4           # one-hot builds batched per DVE op
N_CORES = 8


class Plan:
    """Host-side graph preprocessing producing uniform per-core schedules."""

    def __init__(self, n_nodes, n_cores, edge_src, edge_dst):
        self.n_cores = n_cores
        shard = -(-n_nodes // (n_cores * P)) * P
        npad = shard * n_cores
        win_src = npad // 4
        self.shard, self.npad, self.win_src = shard, npad, win_src
        self.nwin = shard // P
        assert win_src % 2 == 0 and win_src - 1 < 32768 and win_src // 2 - 1 < 32768

        deg = np.bincount(edge_dst, minlength=n_nodes) + 1
        self.dis = np.zeros(npad, np.float32)
        self.dis[:n_nodes] = (1.0 / np.sqrt(deg.astype(np.float64))).astype(np.float32)

        loops = np.arange(n_nodes, dtype=np.int64)
        src = np.concatenate([edge_src, loops])
        dst = np.concatenate([edge_dst, loops])

        core = dst // shard
        w = (dst % shard) // P
        q = src // win_src
        order = np.lexsort((dst, q, core))
        src, dst, core, q, w = src[order], dst[order], core[order], q[order], w[order]

        cnt = np.zeros((n_cores, 4, self.nwin), np.int64)
        np.add.at(cnt, (core, q, w), 1)
        self.pwq = (-(-cnt.max(axis=0) // P) * P).astype(np.int64)   # [4, nwin]
        self.stream_off = np.zeros((4, self.nwin), np.int64)
        for qq in range(4):
            self.stream_off[qq] = np.cumsum(self.pwq[qq]) - self.pwq[qq]
        self.stream_len = self.pwq.sum(axis=1)
        self.chunks = []
        for qq in range(4):
            L, ch = int(self.stream_len[qq]), []
            while L > 0:
                s = min(GCHUNK, L)
                ch.append(s)
                L -= s
            self.chunks.append(ch)

        self.per_core = []
        for c in range(n_cores):
            m = core == c
            sc, dc, qc, wc = src[m], dst[m], q[m], w[m]
            run_key = qc * self.nwin + wc
            starts = np.flatnonzero(np.diff(run_key, prepend=-1))
            rank = np.arange(len(run_key)) - np.repeat(
                np.arange(len(run_key))[starts],
                np.diff(starts, append=len(run_key)))
            pos = self.stream_off[qc, wc] + rank
            self.per_core.append((sc, dc, qc, wc, pos))

        tot_cols = int(self.pwq.sum()) // P
        self.ne1 = -(-tot_cols // 128) * 128
        self.ne2 = -(-(2 * tot_cols) // 128) * 128

    def core_arrays(self, c):
        """idx streams (int16) and schedule-ordered dstid arrays for core c."""
        sc, dc, qc, wc, pos = self.per_core[c]
        shard, win_src = self.shard, self.win_src
        idx1, idx2, d1s, d2es, d2os = [], [], [], [], []
        for q in range(4):
            L = int(self.stream_len[q])
            i1 = np.zeros(L, np.int16)
            i2 = np.zeros(L, np.int16)
            d1 = np.full(L, -1.0, np.float32)
            d2e = np.full(L, -1.0, np.float32)
            d2o = np.full(L, -1.0, np.float32)
            m = qc == q
            p_, s_, d_ = pos[m], sc[m], dc[m]
            # pi-permute layer-1 table rows: v -> g*512 + (v%128)*4 + (v%512)//128
            g_ = s_ // 512
            pp = s_ % 128
            ss = (s_ % 512) // 128
            s1p = g_ * 512 + pp * 4 + ss
            i1[p_] = (s1p - q * win_src).astype(np.int16)
            i2[p_] = ((s_ >> 1) - q * (win_src // 2)).astype(np.int16)
            local = (d_ - c * shard - wc[m] * P).astype(np.float32)
            d1[p_] = local
            even = (s_ % 2) == 0
            d2e[p_[even]] = local[even]
            d2o[p_[~even]] = local[~even]
            idx1.append(i1); idx2.append(i2)
            d1s.append(d1); d2es.append(d2e); d2os.append(d2o)

        a1, a2 = [], []
        for w in range(self.nwin):
            for q in range(4):
                o = int(self.stream_off[q, w])
                n = int(self.pwq[q, w])
                a1.append(d1s[q][o:o + n].reshape(-1, P))
                se = d2es[q][o:o + n].reshape(-1, P)
                so = d2os[q][o:o + n].reshape(-1, P)
                a2.append(np.stack([se, so], axis=1).reshape(-1, P))
        a1 = np.concatenate(a1)
        a2 = np.concatenate(a2)
        a1 = np.concatenate([a1, np.full((self.ne1 - len(a1), P), -1.0, np.float32)])
        a2 = np.concatenate([a2, np.full((self.ne2 - len(a2), P), -1.0, np.float32)])
        return idx1, idx2, a1.T.copy(), a2.T.copy()


def _wrap16(a):
    n = a.shape[0]
    assert n % 16 == 0
    w = a.reshape(n // 16, 16).T.astype(np.int16)
    return np.tile(w, (8, 1))


def build_program(plan, n_cores, rep=1, n_ag=1, stages=('g1','l1','ag','l2')):
    """rep>1 builds a timing variant: n_ag AllGathers up front, then the full
    pipeline (minus AllGather) repeated `rep` times inside a For_i loop."""
    import concourse.bass as bass
    import concourse.bacc as bacc
    import concourse.mybir as mybir
    import concourse.tile as tile

    shard, npad, nwin, win_src = plan.shard, plan.npad, plan.nwin, plan.win_src
    f32 = mybir.dt.float32
    i16 = mybir.dt.int16
    Copy = mybir.ActivationFunctionType.Copy

    nc = bacc.Bacc("TRN2", target_bir_lowering=False, debug=False,
                   enable_asserts=False, num_devices=n_cores,
                   num_swdge_queues=4)

    XB = 3584 if npad % 3584 == 0 else 512
    nxb = npad // XB
    xT = nc.dram_tensor("xT", [nxb * P, XB], f32, kind="ExternalInput")
    disG = nc.dram_tensor("disG", [nxb * P, XB // P], f32, kind="ExternalInput")
    W1 = nc.dram_tensor("W1", [P, CH1], f32, kind="ExternalInput")
    Wc = nc.dram_tensor("Wc", [CH1, CH2], f32, kind="ExternalInput")
    b1r = nc.dram_tensor("b1r", [P, CH1], f32, kind="ExternalInput")
    bcr = nc.dram_tensor("bcr", [P, CH2], f32, kind="ExternalInput")
    iot = nc.dram_tensor("iot", [P, P], f32, kind="ExternalInput")
    idn = nc.dram_tensor("idn", [P, P], f32, kind="ExternalInput")
    disW = nc.dram_tensor("disW", [P, nwin], f32, kind="ExternalInput")
    idx1 = [nc.dram_tensor(f"idx1_{q}", [len(plan.chunks[q]) * P, GCHUNK // 16],
                           i16, kind="ExternalInput") for q in range(4)]
    idx2 = [nc.dram_tensor(f"idx2_{q}", [len(plan.chunks[q]) * P, GCHUNK // 16],
                           i16, kind="ExternalInput") for q in range(4)]
    did1 = nc.dram_tensor("did1", [(plan.ne1 // 128) * P, 128], f32,
                          kind="ExternalInput")
    did2 = nc.dram_tensor("did2", [(plan.ne2 // 128) * P, 128], f32,
                          kind="ExternalInput")
    out_c = nc.dram_tensor("out_c", [shard, CH2], f32, kind="ExternalOutput")

    g1w = [nc.dram_tensor(f"g1w{q}", [win_src, CH1], f32, kind="Internal")
           for q in range(4)]
    g2c = nc.dram_tensor("g2c", [shard, CH2], f32, kind="Internal")
    g2f = nc.dram_tensor("g2f", [npad, CH2], f32, kind="Internal")

    GB = 512
    n_groups = win_src // GB
    assert win_src % GB == 0

    with tile.TileContext(nc) as tc:
        with (
            tc.tile_pool(name="const", bufs=1) as cpool,
            tc.tile_pool(name="xload", bufs=1) as xpool,
            tc.tile_pool(name="stage", bufs=1) as spool,
            tc.tile_pool(name="gat", bufs=1) as gpool,
            tc.tile_pool(name="oh", bufs=1) as ohpool,
            tc.tile_pool(name="didp", bufs=1) as dpool,
            tc.tile_pool(name="ixp", bufs=1) as ipool,
            tc.tile_pool(name="epi", bufs=1) as epool,
            tc.tile_pool(name="ps", bufs=1, space="PSUM") as pspool,
        ):
            W1_t = cpool.tile([P, CH1], f32, name="W1_t")
            Wc_t = cpool.tile([CH1, CH2], f32, name="Wc_t")
            b1_t = cpool.tile([P, CH1], f32, name="b1_t")
            bc_t = cpool.tile([P, CH2], f32, name="bc_t")
            io_t = cpool.tile([P, P], f32, name="io_t")
            id_t = cpool.tile([P, P], f32, name="id_t")
            dW_t = cpool.tile([P, nwin], f32, name="dW_t")
            for t, d in ((W1_t, W1), (Wc_t, Wc), (b1_t, b1r), (bc_t, bcr),
                         (io_t, iot), (id_t, idn), (dW_t, disW)):
                nc.sync.dma_start(out=t[:], in_=d[:, :])

            g2f_pairs = g2f[:, :].rearrange("(a b) c -> a (b c)", b=2)

            def emit_g1_build(level=2):
                for b in range(nxb):
                    xt = xpool.tile([P, XB], f32, tag="xt", bufs=2, name="xt")
                    nc.sync.dma_start(out=xt[:], in_=xT[b * P:(b + 1) * P, :])
                    if level == 0:
                        continue
                    dg = xpool.tile([P, XB // P], f32, tag="dg", bufs=2, name="dg")
                    nc.sync.dma_start(out=dg[:], in_=disG[b * P:(b + 1) * P, :])
                    for g in range(XB // GB):
                        st = spool.tile([P, (GB // P) * CH1], f32, tag="st",
                                        bufs=4, name="st")
                        ps = pspool.tile([P, (GB // P) * CH1], f32, tag="gx",
                                         bufs=2, name="gx")
                        for s in range(GB // P):
                            nc.tensor.matmul(out=ps[:, s * CH1:(s + 1) * CH1],
                                             lhsT=xt[:, (g * 4 + s) * P:
                                                     (g * 4 + s + 1) * P],
                                             rhs=W1_t[:], start=True, stop=True)
                        dslice = dg[:, g * 4:(g + 1) * 4]
                        nc.vector.tensor_tensor(
                            out=st[:].rearrange("p (s c) -> p s c", c=CH1),
                            in0=ps[:].rearrange("p (s c) -> p s c", c=CH1),
                            in1=dslice.rearrange("p (s c) -> p s c", c=1
                                                 ).to_broadcast([P, 4, CH1]),
                            op=mybir.AluOpType.mult)
                        gg = b * (XB // GB) + g
                        qq, gl = gg // n_groups, gg % n_groups
                        if level >= 2 or gg % 8 == 0:
                            nc.sync.dma_start(
                                out=g1w[qq][gl * GB:(gl + 1) * GB, :].rearrange(
                                    "(p s) c -> p s c", s=4),
                                in_=st[:].rearrange("p (s c) -> p s c", c=CH1))

            def propagate(layer):
                idxs = idx1 if layer == 1 else idx2
                did = did1 if layer == 1 else did2
                n_ent = plan.ne1 if layer == 1 else plan.ne2
                gt_cur = [None] * 4
                ch_next = [0] * 4
                ch_start = [0] * 4
                dstate = {"e0": -(10 ** 9), "tile": None, "oh": None}

                def get_gather(q, pos):
                    while True:
                        cur = gt_cur[q]
                        if cur is not None and cur[0] <= pos < cur[1]:
                            return cur
                        s0 = ch_start[q]
                        n = plan.chunks[q][ch_next[q]]
                        ck = ch_next[q]
                        ix = ipool.tile([P, GCHUNK // 16], i16,
                                        tag=f"ix{layer}{q}", bufs=4,
                                        name=f"ix{layer}{q}")
                        nc.sync.dma_start(out=ix[:],
                                          in_=idxs[q][ck * P:(ck + 1) * P, :])
                        gt = gpool.tile([P, (GCHUNK // P) * CH1], f32,
                                        tag=f"g{q}", bufs=4, name=f"g{q}")
                        src_ap = g1w[q][:] if layer == 1 else \
                            g2f_pairs[q * (win_src // 2):(q + 1) * (win_src // 2), :]
                        nc.gpsimd.dma_gather(
                            out_ap=gt[:, :(n // P) * CH1].rearrange(
                                "p (n c) -> p n c", c=CH1),
                            in_ap=src_ap, idxs_ap=ix[:, :n // 16],
                            num_idxs=n, num_idxs_reg=n, elem_size=CH1,
                            single_packet=False, queue_num=q)
                        gt_cur[q] = (s0, s0 + n, gt)
                        ch_start[q] += n
                        ch_next[q] += 1

                def get_oh(entry):
                    if entry % OHB == 0:
                        if not (dstate["e0"] <= entry < dstate["e0"] + 128):
                            e0 = entry - entry % 128
                            dt = dpool.tile([P, 128], f32, tag=f"dt{layer}",
                                            bufs=3, name=f"dt{layer}")
                            tt = e0 // 128
                            nc.sync.dma_start(out=dt[:],
                                              in_=did[tt * P:(tt + 1) * P, :])
                            dstate["e0"], dstate["tile"] = e0, dt
                        dt = dstate["tile"]
                        k = entry - dstate["e0"]
                        oh = ohpool.tile([P, OHB * P], f32, tag="oh", bufs=6,
                                         name="oh")
                        iota_b = io_t[:].rearrange(
                            "p (a c) -> p a c", a=1).to_broadcast([P, OHB, P])
                        did_b = dt[:, k:k + OHB].rearrange(
                            "p (b c) -> p b c", c=1).to_broadcast([P, OHB, P])
                        nc.vector.tensor_tensor(
                            out=oh[:].rearrange("p (b c) -> p b c", c=P),
                            in0=iota_b, in1=did_b, op=mybir.AluOpType.is_equal)
                        dstate["oh"] = oh
                    k = entry % OHB
                    return dstate["oh"][:, k * P:(k + 1) * P]

                entry = 0
                for w in range(nwin):
                    ps = pspool.tile([P, CH1], f32, tag="win", bufs=3, name="win")
                    n_mm = (int(plan.pwq[:, w].sum()) // P) * (2 if layer == 2 else 1)
                    mm = 0
                    for q in range(4):
                        base_pos = int(plan.stream_off[q, w])
                        for cc in range(int(plan.pwq[q, w]) // P):
                            pos = base_pos + cc * P
                            s0, _s1, gt = get_gather(q, pos)
                            j = (pos - s0) // P
                            if layer == 1:
                                oh = get_oh(entry); entry += 1
                                nc.tensor.matmul(
                                    out=ps[:], lhsT=oh,
                                    rhs=gt[:, j * CH1:(j + 1) * CH1],
                                    start=(mm == 0), stop=(mm == n_mm - 1))
                                mm += 1
                            else:
                                ohe = get_oh(entry); entry += 1
                                oho = get_oh(entry); entry += 1
                                nc.tensor.matmul(
                                    out=ps[:, :CH2], lhsT=ohe,
                                    rhs=gt[:, j * CH1:j * CH1 + CH2],
                                    start=(mm == 0), stop=False)
                                mm += 1
                                nc.tensor.matmul(
                                    out=ps[:, :CH2], lhsT=oho,
                                    rhs=gt[:, j * CH1 + CH2:(j + 1) * CH1],
                                    start=False, stop=(mm == n_mm - 1))
                                mm += 1
                    yield w, ps

            def emit_l1():
                for w, ps in propagate(1):
                    t1 = epool.tile([P, CH1], f32, tag="t1", bufs=3, name="t1")
                    nc.scalar.activation(out=t1[:], in_=ps[:], func=Copy,
                                         scale=dW_t[:, w:w + 1])
                    t2 = epool.tile([P, CH1], f32, tag="t2", bufs=3, name="t2")
                    nc.vector.tensor_tensor(out=t2[:], in0=t1[:], in1=b1_t[:],
                                            op=mybir.AluOpType.add)
                    h = epool.tile([P, CH1], f32, tag="h", bufs=3, name="h")
                    nc.vector.tensor_scalar_max(out=h[:], in0=t2[:], scalar1=0.0)
                    pt = pspool.tile([CH1, P], f32, tag="tp", bufs=1, name="tp")
                    nc.tensor.transpose(out=pt[:], in_=h[:], identity=id_t[:])
                    hT = epool.tile([CH1, P], f32, tag="hT", bufs=3, name="hT")
                    nc.vector.tensor_copy(out=hT[:], in_=pt[:])
                    pg = pspool.tile([P, CH2], f32, tag="mm2", bufs=2, name="mm2")
                    nc.tensor.matmul(out=pg[:], lhsT=hT[:], rhs=Wc_t[:],
                                     start=True, stop=True)
                    g2s = epool.tile([P, CH2], f32, tag="g2s", bufs=3, name="g2s")
                    nc.scalar.activation(out=g2s[:], in_=pg[:], func=Copy,
                                         scale=dW_t[:, w:w + 1])
                    nc.sync.dma_start(out=g2c[w * P:(w + 1) * P, :], in_=g2s[:])

            def emit_ag():
                nc.gpsimd.collective_compute(
                    "AllGather", mybir.AluOpType.bypass,
                    replica_groups=[list(range(n_cores))],
                    ins=[g2c[:, :]], outs=[g2f[:, :]])

            def emit_l2():
                for w, ps in propagate(2):
                    o1 = epool.tile([P, CH2], f32, tag="o1", bufs=3, name="o1")
                    nc.scalar.activation(out=o1[:], in_=ps[:, :CH2], func=Copy,
                                         scale=dW_t[:, w:w + 1])
                    o2 = epool.tile([P, CH2], f32, tag="o2", bufs=3, name="o2")
                    nc.vector.tensor_tensor(out=o2[:], in0=o1[:], in1=bc_t[:],
                                            op=mybir.AluOpType.add)
                    nc.sync.dma_start(out=out_c[w * P:(w + 1) * P, :], in_=o2[:])

            if rep == 1:
                if 'g1' in stages: emit_g1_build()
                if 'l1' in stages: emit_l1()
                if 'ag' in stages: emit_ag()
                if 'l2' in stages: emit_l2()
            else:
                for _ in range(n_ag):
                    if 'ag' in stages: emit_ag()
                if any(t in stages for t in ('g1', 'g1d', 'g1m', 'l1', 'l2')):
                    with tc.For_i(0, rep, 1) as _i:
                        if 'g1d' in stages: emit_g1_build(0)
                        if 'g1m' in stages: emit_g1_build(1)
                        if 'g1' in stages: emit_g1_build()
                        if 'l1' in stages: emit_l1()
                        if 'l2' in stages: emit_l2()

    nc.compile()
    return nc


def make_in_maps(plan, x, W1, b1, W_mu, b_mu, W_ls, b_ls):
    n_nodes = np.asarray(x).shape[0]
    npad, shard = plan.npad, plan.shard
    xTfull = np.zeros((P, npad), np.float32)
    xTfull[:, :n_nodes] = np.asarray(x, np.float32).T
    XB = 3584 if npad % 3584 == 0 else 512
    nxb = npad // XB
    xTf = xTfull.reshape(P, nxb, XB).transpose(1, 0, 2).reshape(nxb * P, XB)
    disGf = plan.dis.reshape(nxb, XB // P, P).transpose(0, 2, 1).reshape(
        nxb * P, XB // P)
    Wc = np.concatenate([np.asarray(W_mu, np.float32),
                         np.asarray(W_ls, np.float32)], axis=1)
    bc = np.concatenate([np.asarray(b_mu, np.float32),
                         np.asarray(b_ls, np.float32)])
    iota = np.tile(np.arange(P, dtype=np.float32), (P, 1))
    ident = np.eye(P, dtype=np.float32)

    def chunk_idx(stream, chunks):
        tiles = []
        s0 = 0
        for n in chunks:
            buf = np.zeros((P, GCHUNK // 16), np.int16)
            buf[:, :n // 16] = _wrap16(stream[s0:s0 + n])
            tiles.append(buf)
            s0 += n
        return np.concatenate(tiles, axis=0)

    def chunk_did(a):  # [P, ne] -> [(ne/128)*P, 128]
        ne = a.shape[1]
        return a.reshape(P, ne // 128, 128).transpose(1, 0, 2).reshape(
            (ne // 128) * P, 128)

    in_maps = []
    for c in range(plan.n_cores):
        i1, i2, a1, a2 = plan.core_arrays(c)
        m = {
            "xT": xTf, "W1": np.asarray(W1, np.float32), "Wc": Wc,
            "b1r": np.tile(np.asarray(b1, np.float32), (P, 1)),
            "bcr": np.tile(bc, (P, 1)), "iot": iota, "idn": ident,
            "disG": disGf,
            "disW": plan.dis[c * shard:(c + 1) * shard].reshape(
                plan.nwin, P).T.copy(),
            "did1": chunk_did(a1),
            "did2": chunk_did(a2),
        }
        for q in range(4):
            m[f"idx1_{q}"] = chunk_idx(i1[q], plan.chunks[q])
            m[f"idx2_{q}"] = chunk_idx(i2[q], plan.chunks[q])
        in_maps.append(m)
    return in_maps


def kernel(x, edge_index, W1, b1, W_mu, b_mu, W_ls, b_ls):
    from concourse import bass_utils

    x = np.asarray(x, np.float32)
    n_nodes = x.shape[0]
    plan = Plan(n_nodes, N_CORES, np.asarray(edge_index[0], np.int64),
                np.asarray(edge_index[1], np.int64))
    nc = build_program(plan, N_CORES)
    in_maps = make_in_maps(plan, x, W1, b1, W_mu, b_mu, W_ls, b_ls)
    res = bass_utils.run_bass_kernel_spmd(nc, in_maps,
                                          core_ids=list(range(N_CORES)))
    out = np.concatenate([res.results[c]["out_c"] for c in range(N_CORES)],
                         axis=0)
    return (out[:n_nodes, :16].copy(), out[:n_nodes, 16:].copy())


def _numpy_ref(x, ei, W1, b1, W_mu, b_mu, W_ls, b_ls):
    n = x.shape[0]
    src = np.concatenate([ei[0], np.arange(n)])
    dst = np.concatenate([ei[1], np.arange(n)])
    deg = np.bincount(dst, minlength=n)
    dis = 1 / np.sqrt(deg)

    def conv(f, W, b):
        g = dis[:, None] * (f @ W)
        acc = np.zeros((n, W.shape[1]))
        np.add.at(acc, dst, g[src])
        return dis[:, None] * acc + b

    h = np.maximum(conv(x, W1, b1), 0)
    return conv(h, W_mu, b_mu), conv(h, W_ls, b_ls)


if __name__ == "__main__":
    rng = np.random.default_rng(0)
    N, E, IC = 2048, 16384, 128
    x = rng.standard_normal((N, IC)).astype(np.float32)
    ei = rng.integers(0, N, size=(2, E)).astype(np.int64)
    W1 = (rng.standard_normal((IC, CH1)) / np.sqrt(IC)).astype(np.float32)
    b1 = (rng.standard_normal(CH1) * 0.1).astype(np.float32)
    W_mu = (rng.standard_normal((CH1, 16)) / 8).astype(np.float32)
    b_mu = (rng.standard_normal(16) * 0.1).astype(np.float32)
    W_ls = (rng.standard_normal((CH1, 16)) / 8).astype(np.float32)
    b_ls = (rng.standard_normal(16) * 0.1).astype(np.float32)

    emu, els = _numpy_ref(x, ei, W1, b1, W_mu, b_mu, W_ls, b_ls)
    amu, als = kernel(x, ei, W1, b1, W_mu, b_mu, W_ls, b_ls)
    for name, e, a in (("mu", emu, amu), ("ls", els, als)):
        rel = np.abs(a - e).max() / (np.abs(e).max() + 1e-9)
        print(f"{name}: rel {rel:.3e}")
